# revision 17
# baseline (speedup 1.0000x reference)
"""HQQ 4-bit quantized linear layer on 8 Trainium2 NeuronCores.

Reference computation:
    W_r = concat([W_q >> 4, W_q & 0xF], axis=0).astype(f32)    # [64, 704512]
    W   = ((W_r - zero) * scale).reshape(11008, 4096)          # [out, in]
    out = x @ W.T + bias                                        # [4, 2048, 11008]

Group structure: group j = r*4096 + k (r in [0,172), k in [0,4096)) supplies
output feature o = i*172 + r (element i in [0,64) of the group) at input
feature k.  So for a fixed o, every k belongs to a different group, and
W[o, k] = (nib[i, j] - zero[j]) * scale[j] with i = o//172, j = (o%172)*4096+k.

Sharding (column-parallel over output features, SPMD-uniform):
  core c owns W_q byte-rows [4c, 4c+4).  Byte-row b holds the high nibble of
  i=b and the low nibble of i=b+32, so core c produces output features
  o in {(4c+ib)*172 + r} (high) and {(32+4c+ib)*172 + r} (low), ib in [0,4),
  r in [0,172): 1376 contiguous-after-gather features per core.  Every core
  runs the identical program; x / scale / zero are replicated.

Device kernel (per core), PE-bound at ~1.17 ms of bf16 matmul:
  - dequantize the 4096x1376 weight shard once into SBUF bf16 with a fused
    2-op pipeline:  t = (wq >> 4) * scale ;  w = t - zero*scale
    split across the DVE (vector) and Pool (gpsimd) engines so the PE is not
    starved at startup.
  - x is staged in [128, 8, 256] f32 groups (single 3D DMA each) and cast
    f32->bf16 on the Activation engine in 2048-wide instructions.
  - supersteps 0 and 1 run the matmul accumulation k-OUTER (all 6 PSUM tiles
    live) so the PE consumes w tiles in dequant production order; remaining
    supersteps run sub-outer as usual.
  - PSUM drain is a single fused (psum*1 + bias) op per chunk; gpsimd drains
    the first two supersteps (vector is still dequantizing), vector the rest.
  - DMA queues: x loads on SP (sync), W/scale/zero/bias on Activation,
    output writes on gpsimd - three independent HW queues.
"""

import os
import sys

for _p in ("/opt/trn_rl_repo",):
    if os.path.isdir(_p) and _p not in sys.path:
        sys.path.insert(0, _p)

import numpy as np

P = 128
IN_F = 4096
OUT_F = 11008
GROUP = 64
R_FULL = 172          # OUT_F // GROUP
IB_FULL = 4           # W_q byte rows per core
N_CORES = 8
NTOK_FULL = 8192      # 4 * 2048


def _chunks(n, maxc=512):
    out = []
    off = 0
    while off < n:
        sz = min(maxc, n - off)
        out.append((off, sz))
        off += sz
    return out


def build_program(KT=32, NSUP=32, SUP=256, IB=IB_FULL, R=R_FULL, num_devices=N_CORES):
    """Build the SPMD bass program. Returns the compiled Bacc object."""
    import concourse.bacc as bacc
    import concourse.bass as bass
    import concourse.mybir as mybir
    import concourse.tile as tile
    from concourse.alu_op_type import AluOpType

    f32 = mybir.dt.float32
    bf16 = mybir.dt.bfloat16
    u16 = mybir.dt.uint16

    K = KT * P
    NTOK = NSUP * SUP
    NSUB = SUP // P
    OHALF = IB * R
    OFULL = 2 * OHALF
    CHUNKS = _chunks(OFULL)
    XG = min(8, KT)        # k-tiles per x staging group
    NXG = KT // XG
    # kt indices dequantized on gpsimd (Pool); rest on vector (DVE).
    # DVE runs the all-16-bit dequant in 2x mode (~1.8us/kt); Pool has no 2x
    # (~5.9us/kt) so it gets a small, late-leaning share.
    GP_KT = set()           # empty: gpsimd activity drops the PE clock ~2.4->2.0GHz
    KOUTER_SS = 2          # number of leading supersteps with k-outer ordering

    nc = bacc.Bacc(
        "TRN2", target_bir_lowering=False, debug=False, num_devices=num_devices
    )

    xt = nc.dram_tensor("xt", [K, NTOK], f32, kind="ExternalInput")
    wq = nc.dram_tensor("wq", [K, OHALF], u16, kind="ExternalInput")
    scale_t = nc.dram_tensor("scale_t", [K, R], f32, kind="ExternalInput")
    zero_t = nc.dram_tensor("zero_t", [K, R], f32, kind="ExternalInput")
    bias = nc.dram_tensor("bias", [OFULL], f32, kind="ExternalInput")
    out = nc.dram_tensor("out", [NTOK, OFULL], f32, kind="ExternalOutput")

    with tile.TileContext(nc) as tc:
        with (
            tc.tile_pool(name="cst", bufs=1) as cst,
            tc.tile_pool(name="wres", bufs=1) as wres,
            tc.tile_pool(name="dq", bufs=3) as dq,
            tc.tile_pool(name="xf", bufs=3) as xfp,
            tc.tile_pool(name="xb", bufs=2) as xbp,
            tc.tile_pool(name="psum", bufs=2, space="PSUM") as pp,
            tc.tile_pool(name="outp", bufs=2) as op,
        ):
            bias_b = cst.tile([P, OFULL], f32)

            # --- x staging helper: [P, XG, SUP] f32 single DMA + wide bf16 cast
            def stage(ns):
                tok0 = ns * SUP
                xb_t = xbp.tile([P, KT, SUP], bf16, tag="xb")
                for g in range(NXG):
                    xf_t = xfp.tile([P, XG, SUP], f32, tag="xf")
                    k0 = g * XG * P
                    src = bass.AP(
                        xt, k0 * NTOK + tok0,
                        [[NTOK, P], [P * NTOK, XG], [1, SUP]],
                    )
                    nc.sync.dma_start(out=xf_t[:], in_=src)
                    nc.scalar.copy(out=xb_t[:, g * XG:(g + 1) * XG, :], in_=xf_t[:])
                return xb_t

            # stage the first two supersteps before anything else so the x
            # pipeline (sync DMA + Activation cast) starts immediately
            xb_tiles = {0: stage(0)}
            if NSUP > 1:
                xb_tiles[1] = stage(1)

            # --- W/scale/zero DMAs, all issued upfront on the Activation queue
            w_in = []
            for kt in range(KT):
                ks = slice(kt * P, (kt + 1) * P)
                wq_t = dq.tile([P, OHALF], u16, tag="wq")
                nc.scalar.dma_start(out=wq_t[:], in_=wq[ks, :])
                sc = dq.tile([P, R], f32, tag="sc")
                nc.scalar.dma_start(out=sc[:], in_=scale_t[ks, :])
                zr = dq.tile([P, R], f32, tag="zr")
                nc.scalar.dma_start(out=zr[:], in_=zero_t[ks, :])
                w_in.append((wq_t, sc, zr))
                if kt == min(7, KT - 1):
                    # bias broadcast to [128, OFULL]; needed at first drain
                    bias_bcast_src = bass.AP(bias, 0, [[0, P], [1, OFULL]])
                    nc.scalar.dma_start(out=bias_b[:], in_=bias_bcast_src)

            # --- dequantize whole shard into resident SBUF bf16 (DVE || Pool)
            # All big ops are 16-bit in/out so the DVE runs them in 2x mode:
            #   nib = wq >> 4 / wq & 15            (u16 -> u16, DVE only)
            #   t   = nib * sc_bf (bcast)          (u16 x bf16 -> bf16)
            #   w   = t - zs_bf (bcast)            (bf16)
            # Pool (gpsimd) supports only tensor_tensor, so DVE does every
            # extraction + the small per-kt scalar prep; the mult/sub pairs
            # are split DVE/Pool.  Pool-destined intermediates get deeper
            # buffers so Pool's lag never throttles the DVE.
            w_res = [
                wres.tile([P, OFULL], bf16, tag=f"w{kt}", name=f"w{kt}")
                for kt in range(KT)
            ]
            for kt in range(KT):
                pool_kt = kt in GP_KT
                eng = nc.gpsimd if pool_kt else nc.vector
                sfx = "p" if pool_kt else ""
                wq_t, sc, zr = w_in[kt]
                # zs_bf = (zr + 0) * sc -> bf16 ; sc_bf = sc -> bf16 (tiny, DVE)
                zs = dq.tile([P, R], bf16, tag=f"zs{sfx}", bufs=8 if pool_kt else 2)
                nc.vector.scalar_tensor_tensor(
                    out=zs[:], in0=zr[:], scalar=0.0, in1=sc[:],
                    op0=AluOpType.add, op1=AluOpType.mult,
                )
                scb = dq.tile([P, R], bf16, tag=f"scb{sfx}", bufs=8 if pool_kt else 2)
                nc.vector.tensor_scalar_add(out=scb[:], in0=sc[:], scalar1=0.0)
                # broadcast [P, R] -> [P, IB, R] with stride-0 middle dim
                sc_b = bass.AP(scb.tensor, scb.offset, [scb.ap[0], [0, IB], [1, R]])
                zs_b = bass.AP(zs.tensor, zs.offset, [zs.ap[0], [0, IB], [1, R]])
                for half, (op0, scl) in enumerate(
                    [(AluOpType.logical_shift_right, 4), (AluOpType.bitwise_and, 15)]
                ):
                    nib = dq.tile([P, OHALF], u16, tag=f"nib{sfx}",
                                  bufs=6 if pool_kt else 3)
                    nc.vector.tensor_scalar(
                        out=nib[:], in0=wq_t[:], scalar1=scl, scalar2=None, op0=op0
                    )
                    t = dq.tile([P, OHALF], bf16, tag=f"t{sfx}")
                    eng.tensor_tensor(out=t[:], in0=nib[:], in1=sc_b, op=AluOpType.mult)
                    eng.tensor_tensor(
                        out=w_res[kt][:, half * OHALF:(half + 1) * OHALF],
                        in0=t[:], in1=zs_b, op=AluOpType.subtract,
                    )

            def drain(ns, sub, ps, eng, final=False):
                ot = op.tile([P, OFULL], f32, tag="ot")
                row0 = ns * SUP + sub * P
                for ci, (off, sz) in enumerate(CHUNKS):
                    eng.scalar_tensor_tensor(
                        out=ot[:, off:off + sz], in0=ps[ci][:], scalar=1.0,
                        in1=bias_b[:, off:off + sz],
                        op0=AluOpType.mult, op1=AluOpType.add,
                    )
                    if final:
                        # pipeline drain->DMA per chunk to shrink the tail
                        nc.sync.dma_start(
                            out=out[row0:row0 + P, off:off + sz],
                            in_=ot[:, off:off + sz],
                        )
                if not final:
                    nc.sync.dma_start(out=out[row0:row0 + P, :], in_=ot[:])

            # --- main GEMM loop ---
            for ns in range(NSUP):
                xb_t = xb_tiles.pop(ns)
                if ns < KOUTER_SS:
                    # k-outer: all NSUB*len(CHUNKS) psum tiles live; PE follows
                    # dequant production order through the first supersteps.
                    ps = {
                        sub: [
                            pp.tile([P, sz], f32, tag=f"ps{ci}", name=f"ps{ci}")
                            for ci, (off, sz) in enumerate(CHUNKS)
                        ]
                        for sub in range(NSUB)
                    }
                    for kt in range(KT):
                        for sub in range(NSUB):
                            lhsT = xb_t[:, kt, sub * P:(sub + 1) * P]
                            for ci, (off, sz) in enumerate(CHUNKS):
                                nc.tensor.matmul(
                                    ps[sub][ci][:], lhsT,
                                    w_res[kt][:, off:off + sz],
                                    start=(kt == 0), stop=(kt == KT - 1),
                                    skip_group_check=True,
                                )
                    for sub in range(NSUB):
                        drain(ns, sub, ps[sub], nc.vector)

                else:
                    for sub in range(NSUB):
                        ps = [
                            pp.tile([P, sz], f32, tag=f"ps{ci}", name=f"ps{ci}")
                            for ci, (off, sz) in enumerate(CHUNKS)
                        ]
                        for kt in range(KT):
                            for ci, (off, sz) in enumerate(CHUNKS):
                                nc.tensor.matmul(
                                    ps[ci][:], xb_t[:, kt, sub * P:(sub + 1) * P],
                                    w_res[kt][:, off:off + sz],
                                    start=(kt == 0), stop=(kt == KT - 1),
                                    skip_group_check=True,
                                )
                        drain(ns, sub, ps, nc.vector,
                              final=(ns == NSUP - 1 and sub == NSUB - 1))
                if ns + 2 < NSUP:
                    xb_tiles[ns + 2] = stage(ns + 2)

    nc.compile()
    return nc


_PROG_CACHE = {}


def _get_program():
    key = "full"
    if key not in _PROG_CACHE:
        _PROG_CACHE[key] = build_program()
    return _PROG_CACHE[key]


def shard_inputs(x, W_q, scale, zero, bias):
    """Host-side sharding / layout transforms (no arithmetic on values)."""
    x = np.asarray(x, dtype=np.float32)
    W_q = np.asarray(W_q)
    scale = np.asarray(scale, dtype=np.float32)
    zero = np.asarray(zero, dtype=np.float32)
    bias = np.asarray(bias, dtype=np.float32)

    ntok = x.shape[0] * x.shape[1]
    xt = np.ascontiguousarray(x.reshape(ntok, IN_F).T)              # [K, NTOK]
    scale_t = np.ascontiguousarray(scale.reshape(R_FULL, IN_F).T)   # [K, R]
    zero_t = np.ascontiguousarray(zero.reshape(R_FULL, IN_F).T)     # [K, R]
    wq_u8 = W_q.astype(np.uint16)                                   # values < 256
    bias_rs = bias.reshape(GROUP, R_FULL)                           # [i, r]

    in_maps = []
    for c in range(N_CORES):
        rows = wq_u8[IB_FULL * c: IB_FULL * (c + 1)]                # [4, 704512]
        # [ib, r, k] -> [k, ib, r] -> [K, OHALF]
        wq_c = np.ascontiguousarray(
            rows.reshape(IB_FULL, R_FULL, IN_F).transpose(2, 0, 1)
        ).reshape(IN_F, IB_FULL * R_FULL)
        bias_c = np.concatenate(
            [
                bias_rs[IB_FULL * c: IB_FULL * (c + 1)].ravel(),
                bias_rs[32 + IB_FULL * c: 32 + IB_FULL * (c + 1)].ravel(),
            ]
        )
        in_maps.append(
            {
                "xt": xt,
                "wq": wq_c,
                "scale_t": scale_t,
                "zero_t": zero_t,
                "bias": bias_c,
            }
        )
    return in_maps


def gather_output(results, ntok=NTOK_FULL):
    out = np.empty((ntok, OUT_F), dtype=np.float32)
    ohalf = IB_FULL * R_FULL
    for c in range(N_CORES):
        res = results[c]["out"]
        lo = IB_FULL * c * R_FULL
        out[:, lo: lo + ohalf] = res[:, :ohalf]
        lo = (32 + IB_FULL * c) * R_FULL
        out[:, lo: lo + ohalf] = res[:, ohalf:]
    return out


def kernel(x, W_q, scale, zero, bias):
    from concourse.bass_utils import run_bass_kernel_spmd

    x = np.asarray(x)
    b, s, _ = x.shape
    nc = _get_program()
    in_maps = shard_inputs(x, W_q, scale, zero, bias)
    res = run_bass_kernel_spmd(nc, in_maps, list(range(N_CORES)))
    out = gather_output(res.results)
    return out.reshape(b, s, OUT_F)
